# revision 7
# baseline (speedup 1.0000x reference)
"""Trainium2 Bass kernel for nn_FBRNN (4-layer GRU stack + per-step additive
attention over layer states + MLP head, T=131072, batch=1).

Strategy: the recurrence is strongly contractive (state influence decays ~2x
per step), so the sequence is split into 8*S independent lanes (S lanes per
core), each processing a contiguous chunk of CHUNK steps after a W-step
warmup whose initial-state error decays below fp32 noise. All lanes of one
core advance in lockstep: every tile holds [features, S-lanes] with features
on SBUF partitions, so the whole per-step computation is small matmuls
(TensorE) + a few wide elementwise ops (VectorE/ScalarE).

The layer-0 input projection (batch @ Wih0.T, the memory-bound part) is
precomputed as a bulk tiled matmul from a host-pretransposed batch slice.
The softmax in the attention combine is computed unnormalized via exp and a
final reciprocal multiply; all gathers / reductions across layer blocks are
expressed as matmuls with constant 0/1 (or replicated-vector) lhsT matrices
built on the host. The MLP head runs as bulk batched matmuls over the saved
top-layer states, off the critical chain.
"""

import numpy as np
from contextlib import ExitStack

import concourse.bass as bass
import concourse.tile as tile
from concourse import bacc, mybir
from concourse import bass_utils

F32 = mybir.dt.float32
AF = mybir.ActivationFunctionType

T, F, HID, L, AH = 131072, 256, 16, 4, 16
NCORES = 8
PER = T // NCORES            # 16384 timesteps per core
S = 256                      # parallel lanes per core
W = 32                       # warmup steps per lane
CHUNK = PER // S             # 64 real steps per lane
NSTEPS = W + CHUNK           # 96
NL = PER + CHUNK             # padded projection columns (lane stride layout)
PAIRS1 = [(0, 0), (0, 1), (0, 2), (0, 3), (1, 1), (1, 2), (1, 3)]
PAIRS2 = [(2, 2), (2, 3), (3, 3)]
NP1, NP2 = 16 * len(PAIRS1), 16 * len(PAIRS2)   # 112, 48

# ---------------------------------------------------------------- constants


def _build_consts(I):
    """Builds every constant matrix the kernel needs, keyed by input name.

    G PSUM layout per layer: rows 0:32 = rz_sum (r|z), 32:48 = gi_n,
    48:64 = gh_n. R PSUM layout: cols 0:64 numerator (16i+h), 64:68 denom_i.
    Attention gather lhsTs are split per source layer j so every matmul
    operand sits at SBUF base partition 0.
    """
    C = {}
    C["projA"] = np.ascontiguousarray(I["Wih0"].T[0:128])      # [128,48]
    C["projB"] = np.ascontiguousarray(I["Wih0"].T[128:256])    # [128,48]
    C["projb"] = I["bih0"].reshape(1, 48).astype(np.float32)
    gi0 = np.zeros((48, 112), np.float32)
    gi0[0:16, 0:16] = np.eye(16)      # r rows -> cols 0:16
    gi0[16:32, 64:80] = np.eye(16)    # z rows -> cols 64:80
    gi0[32:48, 32:48] = np.eye(16)    # n rows -> gi_n cols 32:48
    C["gi0_lhsT"] = gi0
    for l in range(L):
        if l == 0:
            Wih, Whh, bih, bhh = I["Wih0"], I["Whh0"], I["bih0"], I["bhh0"]
        else:
            Wih, Whh, bih, bhh = (I["Wih"][l - 1], I["Whh"][l - 1],
                                  I["bih"][l - 1], I["bhh"][l - 1])
        ih = np.zeros((16, 112), np.float32)
        if l > 0:
            ih[:, 0:16] = Wih[0:16].T
            ih[:, 64:80] = Wih[16:32].T
            ih[:, 32:48] = Wih[32:48].T
        hh = np.zeros((16, 112), np.float32)
        hh[:, 0:16] = Whh[0:16].T
        hh[:, 64:80] = Whh[16:32].T
        hh[:, 96:112] = Whh[32:48].T
        b = np.zeros((1, 112), np.float32)
        if l == 0:
            b[0, 0:16] = bhh[0:16]          # bih0 folded into projection
            b[0, 64:80] = bhh[16:32]
            b[0, 96:112] = bhh[32:48]
        else:
            b[0, 0:16] = bih[0:16] + bhh[0:16]
            b[0, 64:80] = bih[16:32] + bhh[16:32]
            b[0, 32:48] = bih[32:48]
            b[0, 96:112] = bhh[32:48]
        C[f"ih{l}"] = ih
        C[f"hh{l}"] = hh
        C[f"b{l}"] = b

    def build_attn(tag, pairs):
        n = len(pairs)
        js = sorted({j for _, j in pairs})
        for j in js:
            lE = np.zeros((16, 16 * n), np.float32)
            lsel = np.zeros((16, 16 * n), np.float32)
            for p, (i, jj) in enumerate(pairs):
                if jj == j:
                    lE[:, 16 * p:16 * p + 16] = I["aW"][i]
                    lsel[:, 16 * p:16 * p + 16] = np.eye(16)
            C[f"E{tag}_{j}"] = lE
            C[f"sel{tag}_{j}"] = lsel
        bab = np.zeros((16 * n, 1), np.float32)
        lav = np.zeros((16 * n, 16 * n), np.float32)
        bavb = np.zeros((16 * n, 1), np.float32)
        lsum = np.zeros((16 * n, 112), np.float32)
        lden = np.zeros((16 * n, 4), np.float32)
        for p, (i, j) in enumerate(pairs):
            bab[16 * p:16 * p + 16, 0] = I["ab"][i]
            lav[16 * p:16 * p + 16, 16 * p:16 * p + 16] = np.repeat(
                I["av"][i][:, None], 16, 1)
            bavb[16 * p:16 * p + 16, 0] = I["avb"][i]
            lsum[16 * p:16 * p + 16, 32 * i:32 * i + 16] = np.eye(16)
            lden[16 * p, i] = 1.0
        C[f"ab{tag}"] = bab
        C[f"av{tag}"] = lav
        C[f"avb{tag}"] = bavb
        C[f"sum{tag}"] = lsum
        C[f"den{tag}"] = lden

    build_attn("1", PAIRS1)
    build_attn("2", PAIRS2)
    rep = np.zeros((4, 112), np.float32)
    for i in range(4):
        rep[i, 32 * i:32 * i + 16] = 1.0
    C["rep"] = rep
    C["fc1"] = np.ascontiguousarray(I["fc1W"].T)     # [16,32]
    C["fc1b"] = I["fc1b"].reshape(32, 1).astype(np.float32)
    C["fc2"] = np.ascontiguousarray(I["fc2W"].T)     # [32,1]
    C["ones"] = np.ones((1, 512), np.float32)
    return {k: np.ascontiguousarray(v, dtype=np.float32) for k, v in C.items()}


_CONST_SHAPES = {
    "projA": (128, 48), "projB": (128, 48), "projb": (1, 48),
    "gi0_lhsT": (48, 112),
    **{f"ih{l}": (16, 112) for l in range(L)},
    **{f"hh{l}": (16, 112) for l in range(L)},
    **{f"b{l}": (1, 112) for l in range(L)},
    **{f"E1_{j}": (16, NP1) for j in range(4)},
    **{f"sel1_{j}": (16, NP1) for j in range(4)},
    **{f"E2_{j}": (16, NP2) for j in (2, 3)},
    **{f"sel2_{j}": (16, NP2) for j in (2, 3)},
    "ab1": (NP1, 1), "av1": (NP1, NP1), "avb1": (NP1, 1),
    "sum1": (NP1, 112), "den1": (NP1, 4),
    "ab2": (NP2, 1), "av2": (NP2, NP2), "avb2": (NP2, 1),
    "sum2": (NP2, 112), "den2": (NP2, 4),
    "rep": (4, 112), "fc1": (16, 32), "fc1b": (32, 1), "fc2": (32, 1),
    "ones": (1, 512),
}

# ---------------------------------------------------------------- program


def emit_kernel(ctx, tc, ins, y):
    nc = tc.nc
    cp = ctx.enter_context(tc.tile_pool(name="consts", bufs=1))

    ct = {}
    for cname, shape in _CONST_SHAPES.items():
        t = cp.tile(list(shape), F32, name=f"c_{cname}")
        nc.sync.dma_start(t[:], ins[cname][:])
        ct[cname] = t
    maskt = cp.tile([16, S], F32)
    nc.sync.dma_start(maskt[:], ins["mask"][:])

    ones = ct["ones"]

    # persistent state: 3 lower-layer attention outputs (ping-pong) + top
    gi0 = cp.tile([48, NL], F32)
    hist = cp.tile([16, PER], F32)
    st = [[cp.tile([16, S], F32, name=f"st{p}_{i}") for i in range(3)]
          for p in range(2)]
    tops = [cp.tile([16, S], F32, name=f"tops{p}") for p in range(2)]
    zero16 = cp.tile([16, S], F32)
    for i in range(3):
        nc.vector.memset(st[0][i][:], 0.0)
    nc.vector.memset(zero16[:], 0.0)

    # ---------------- phase A: bulk input projection gi0 = Wih0 @ x + bih0
    with tc.tile_pool(name="bstage", bufs=4) as bstage, \
            tc.tile_pool(name="ppsum", bufs=2, space="PSUM") as ppsum:
        off = 0
        while off < NL:
            n = min(512, NL - off)
            ba = bstage.tile([128, 512], F32, tag="ba")
            nc.sync.dma_start(ba[:, 0:n], ins["bTa"][:, off:off + n])
            bb = bstage.tile([128, 512], F32, tag="bb")
            nc.sync.dma_start(bb[:, 0:n], ins["bTb"][:, off:off + n])
            ps = ppsum.tile([48, 512], F32)
            nc.tensor.matmul(ps[:, 0:n], ct["projA"][:], ba[:, 0:n],
                             start=True, stop=False)
            nc.tensor.matmul(ps[:, 0:n], ct["projB"][:], bb[:, 0:n],
                             start=False, stop=False)
            nc.tensor.matmul(ps[:, 0:n], ct["projb"][:], ones[:, 0:n],
                             start=False, stop=True)
            nc.scalar.activation(gi0[:, off:off + n], ps[:, 0:n], AF.Copy)
            off += n

    gi0v = gi0[:].rearrange("p (c s) -> p c s", s=CHUNK)   # [48, S+1, CHUNK]

    # ---------------- phase B: recurrence
    gpool = ctx.enter_context(tc.tile_pool(name="gpsum", bufs=2, space="PSUM"))
    apsum = ctx.enter_context(tc.tile_pool(name="apsum", bufs=1, space="PSUM"))
    rpsum = ctx.enter_context(tc.tile_pool(name="rpsum", bufs=1, space="PSUM"))
    ew = ctx.enter_context(tc.tile_pool(name="ew", bufs=2))
    mpool = ctx.enter_context(tc.tile_pool(name="mpool", bufs=1))
    hnpool = ctx.enter_context(tc.tile_pool(name="hn", bufs=2))

    def top_slot(sp):
        if sp < 0:
            return zero16[:]
        if sp < W:
            return tops[sp % 2][:]
        return hist[:, (sp - W) * S:(sp - W + 1) * S]

    for sp in range(NSTEPS):
        st_cur = [t[:] for t in st[sp % 2]]
        st_nxt = [t[:] for t in st[(sp + 1) % 2]]
        top_prev = top_slot(sp - 1)
        if sp == W:
            masked = []
            for i in range(3):
                m = mpool.tile([16, S], F32, tag=f"mask{i}")
                nc.vector.tensor_mul(m[:], st_cur[i], maskt[:])
                masked.append(m[:])
            st_cur = masked
            mt = mpool.tile([16, S], F32, tag="mask3")
            nc.vector.tensor_mul(mt[:], top_prev, maskt[:])
            top_prev = mt[:]

        q, r = divmod(sp, CHUNK)
        hn = []
        for l in range(L):
            G = gpool.tile([112, S], F32)
            if l == 0:
                nc.tensor.matmul(G[:], ct["gi0_lhsT"][:],
                                 gi0v[:, q:q + S, r].opt(),
                                 start=True, stop=False)
            else:
                nc.tensor.matmul(G[:], ct[f"ih{l}"][:], hn[l - 1],
                                 start=True, stop=False)
            state = st_cur[l] if l < 3 else top_prev
            nc.tensor.matmul(G[:], ct[f"hh{l}"][:], state,
                             start=False, stop=False)
            nc.tensor.matmul(G[:], ct[f"b{l}"][:], ones[:, 0:S],
                             start=False, stop=True)
            rt = ew.tile([16, S], F32, tag="rt")
            nc.scalar.activation(rt[:], G[0:16, :], AF.Sigmoid)
            zt = ew.tile([16, S], F32, tag="zt")
            nc.scalar.activation(zt[:], G[64:80, :], AF.Sigmoid)
            t1 = ew.tile([16, S], F32, tag="t1")
            nc.vector.tensor_mul(t1[:], rt[:], G[96:112, :])
            t2 = ew.tile([16, S], F32, tag="t2")
            nc.vector.tensor_add(t2[:], t1[:], G[32:48, :])
            nt = ew.tile([16, S], F32, tag="nt")
            nc.scalar.activation(nt[:], t2[:], AF.Tanh)
            dt = ew.tile([16, S], F32, tag="dt")
            nc.vector.tensor_sub(dt[:], state, nt[:])
            t3 = ew.tile([16, S], F32, tag="t3")
            nc.vector.tensor_mul(t3[:], zt[:], dt[:])
            h = hnpool.tile([16, S], F32, tag=f"hn{l}")
            nc.vector.tensor_add(h[:], t3[:], nt[:])
            hn.append(h[:])

        # attention combine (per-source-layer accumulating gather matmuls)
        e1p = apsum.tile([NP1, S], F32, tag="a1")
        for j in range(4):
            nc.tensor.matmul(e1p[:], ct[f"E1_{j}"][:], hn[j],
                             start=(j == 0), stop=(j == 3))
        e2p = apsum.tile([NP2, S], F32, tag="a2")
        for j in (2, 3):
            nc.tensor.matmul(e2p[:], ct[f"E2_{j}"][:], hn[j],
                             start=(j == 2), stop=(j == 3))
        e1 = ew.tile([NP1, S], F32, tag="e1")
        nc.scalar.activation(e1[:], e1p[:], AF.Tanh, bias=ct["ab1"][:])
        e2 = ew.tile([NP2, S], F32, tag="e2")
        nc.scalar.activation(e2[:], e2p[:], AF.Tanh, bias=ct["ab2"][:])
        s1p = apsum.tile([NP1, S], F32, tag="a1")
        nc.tensor.matmul(s1p[:], ct["av1"][:], e1[:], start=True, stop=True)
        s2p = apsum.tile([NP2, S], F32, tag="a2")
        nc.tensor.matmul(s2p[:], ct["av2"][:], e2[:], start=True, stop=True)
        a1 = ew.tile([NP1, S], F32, tag="aa1")
        nc.scalar.activation(a1[:], s1p[:], AF.Exp, bias=ct["avb1"][:])
        a2 = ew.tile([NP2, S], F32, tag="aa2")
        nc.scalar.activation(a2[:], s2p[:], AF.Exp, bias=ct["avb2"][:])
        hg1 = apsum.tile([NP1, S], F32, tag="a1")
        for j in range(4):
            nc.tensor.matmul(hg1[:], ct[f"sel1_{j}"][:], hn[j],
                             start=(j == 0), stop=(j == 3))
        hg2 = apsum.tile([NP2, S], F32, tag="a2")
        for j in (2, 3):
            nc.tensor.matmul(hg2[:], ct[f"sel2_{j}"][:], hn[j],
                             start=(j == 2), stop=(j == 3))
        p1 = ew.tile([NP1, S], F32, tag="p1")
        nc.vector.tensor_mul(p1[:], a1[:], hg1[:])
        p2 = ew.tile([NP2, S], F32, tag="p2")
        nc.vector.tensor_mul(p2[:], a2[:], hg2[:])
        Rn = rpsum.tile([112, S], F32)
        nc.tensor.matmul(Rn[:], ct["sum1"][:], p1[:], start=True, stop=False)
        nc.tensor.matmul(Rn[:], ct["sum2"][:], p2[:], start=False, stop=True)
        Rd = apsum.tile([4, S], F32, tag="a2")
        nc.tensor.matmul(Rd[:], ct["den1"][:], a1[:], start=True, stop=False)
        nc.tensor.matmul(Rd[:], ct["den2"][:], a2[:], start=False, stop=True)
        rc = ew.tile([4, S], F32, tag="rc")
        nc.vector.reciprocal(rc[:], Rd[:])
        rcrep = apsum.tile([112, S], F32, tag="a1")
        nc.tensor.matmul(rcrep[:], ct["rep"][:], rc[:], start=True, stop=True)
        num = ew.tile([112, S], F32, tag="num")
        nc.scalar.activation(num[:], Rn[:], AF.Copy)
        for i in range(3):
            nc.vector.tensor_mul(st_nxt[i], num[32 * i:32 * i + 16, :],
                                 rcrep[32 * i:32 * i + 16, :])
        nc.vector.tensor_mul(top_slot(sp), num[96:112, :], rcrep[96:112, :])

    # ---------------- phase C: head over saved top-layer states
    hpsum = ctx.enter_context(tc.tile_pool(name="hpsum", bufs=1, space="PSUM"))
    opool = ctx.enter_context(tc.tile_pool(name="opool", bufs=3))
    for jb in range(PER // 512):
        cols = slice(jb * 512, (jb + 1) * 512)
        f1 = hpsum.tile([32, 512], F32, tag="f1")
        nc.tensor.matmul(f1[:], ct["fc1"][:], hist[:, cols],
                         start=True, stop=True)
        rl = opool.tile([32, 512], F32, tag="rl")
        nc.scalar.activation(rl[:], f1[:], AF.Relu, bias=ct["fc1b"][:])
        f2 = hpsum.tile([1, 512], F32, tag="f2")
        nc.tensor.matmul(f2[:], ct["fc2"][:], rl[:], start=True, stop=True)
        ot = opool.tile([1, 512], F32, tag="ot")
        nc.scalar.activation(ot[:], f2[:], AF.Copy)
        nc.sync.dma_start(y[:, cols], ot[:])


def build_program():
    nc = bacc.Bacc("TRN2", target_bir_lowering=False, debug=False,
                   enable_asserts=False, num_devices=NCORES)
    ins = {}

    def dram_in(name, shape):
        ins[name] = nc.dram_tensor(name, list(shape), F32,
                                   kind="ExternalInput").ap()

    dram_in("bTa", (128, NL))
    dram_in("bTb", (128, NL))
    dram_in("mask", (16, S))
    for cname, shape in _CONST_SHAPES.items():
        dram_in(cname, shape)
    y = nc.dram_tensor("y", [1, PER], F32, kind="ExternalOutput").ap()
    with tile.TileContext(nc) as tc:
        with ExitStack() as ctx:
            emit_kernel(ctx, tc, ins, y)
    nc.compile()
    return nc


_NC_CACHE = {}


def make_in_maps(inputs):
    I = {k: np.ascontiguousarray(np.asarray(v), dtype=np.float32)
         for k, v in inputs.items()}
    C = _build_consts(I)
    batch = I["batch"]            # [T, 1, F]
    in_maps = []
    for core in range(NCORES):
        start = core * PER - W
        idx = (np.arange(NL) + start) % T
        bT = np.ascontiguousarray(batch[idx, 0, :].T)   # [256, NL]
        mask = np.ones((16, S), np.float32)
        if core == 0:
            mask[:, 0] = 0.0
        m = {"bTa": np.ascontiguousarray(bT[0:128]),
             "bTb": np.ascontiguousarray(bT[128:256]),
             "mask": mask}
        m.update(C)
        in_maps.append(m)
    return in_maps, float(I["fc2b"][0])


def assemble_output(results, fc2b):
    ys = []
    for core in range(NCORES):
        yc = results[core]["y"].reshape(CHUNK, S)
        ys.append(yc.T.reshape(-1))      # lane-major -> timestep order
    y = np.concatenate(ys) + fc2b
    return y.reshape(T, 1).astype(np.float32)


def kernel(**inputs):
    if "nc" not in _NC_CACHE:
        _NC_CACHE["nc"] = build_program()
    nc = _NC_CACHE["nc"]
    in_maps, fc2b = make_in_maps(inputs)
    res = bass_utils.run_bass_kernel_spmd(nc, in_maps,
                                          core_ids=list(range(NCORES)))
    return assemble_output(res.results, fc2b)
